# revision 1
# baseline (speedup 1.0000x reference)
"""Trainium2 Bass kernel for the vq_codebook / HDC problem.

Math (reference):
    hv      = sign(feat @ proj_w.T)                  [N=16384, D=10000], +-1 (0 -> +1)
    per_cls = segment_sum(hv, labels, K=3)           [3, D]
    updated = classify_weights + 0.5 * per_cls
    protos  = updated / max(||updated||_row, eps)
    logits  = hv @ protos.T                          [N, 3]

Strategy (8 NeuronCores):
  * Shard along D: each core owns 1250 hyper-dims, all N rows. Per-class
    sums are then fully core-local (no mid-kernel collective at all).
  * Host sorts rows by label, so segment sums become contiguous-range sums
    along the free (n) axis -- fused into the sign op via accum_out.
  * Device computes, per core:  hvT = sign(projwT_loc.T-tiles @ featT)  in
    [d, n] layout (fp32r matmul, 1 cyc/row), sign via ACT(Sign)+accum /
    DVE(is_ge*2 then -1)+accum, hv stored bf16 in SBUF (never HBM).
    Then u2 = 2*CW_loc + S_loc (bf16) and P2 = u2-tiles.T @ hvT  ->
    [3, N] partial (un-normalized 2*logits contribution of this d-slice).
  * hv SBUF residency is grouped (d-tile groups [3,3,4]); P2 partials per
    group go straight to DRAM; host sums 8 cores x 3 groups, applies the
    1/(2*norm) scale and un-permutes rows.  All heavy lifting (>99.99% of
    FLOPs and bytes) is on-device; host does only O(N*K + D*K) assembly.
"""

import os
import sys

sys.path.insert(0, "/opt/trn_rl_repo")
os.environ.setdefault("MYCRO_LOCAL_CACHE", "1")

import numpy as np

import concourse.bass as bass
import concourse.tile as tile
from concourse import bacc
from concourse import mybir
from concourse.bass import MemorySpace
from concourse.bass_utils import run_bass_kernel_spmd

# ---------------------------------------------------------------- constants
N = 16384          # rows
C = 128            # feat dim (contraction)
D = 10000          # hyper dim
K = 3              # classes
NCORES = 8
DLOC = D // NCORES          # 1250 per core
PT = 125                    # partitions per d-tile
NT = DLOC // PT             # 10 d-tiles per core
NCH = 512                   # n-chunk (matmul moving free size)
NJ = N // NCH               # 32 chunks
PCH = 1024                  # P2 psum superchunk
NJJ = N // PCH              # 16
GROUPS = [[0, 1, 2], [3, 4, 5], [6, 7, 8, 9]]
NG = len(GROUPS)
HV_BUFS = 5                 # SBUF slots for [PT, N] bf16 hv tiles (32KB/part each)
ACT_MOD = 4                 # sign tiles: ACT engine unless idx % ACT_MOD == ACT_MOD-1
MM_DT = mybir.dt.float32r   # encode-matmul dtype (1 cyc/row vs 4 for fp32)

LAM = 0.5
EPS = 1e-12

LAST_RESULTS = None         # BassKernelResults of the most recent run (for test.py)


def _chunk_segments(j, cuts):
    """Segments of chunk [j*NCH, (j+1)*NCH) split at sorted-label boundaries.

    Returns [(s0, s1, cls)] with s0/s1 chunk-relative."""
    lo, hi = j * NCH, (j + 1) * NCH
    pts = [lo] + [b for b in cuts if lo < b < hi] + [hi]
    segs = []
    for a, b in zip(pts[:-1], pts[1:]):
        cls = 0 if a < cuts[0] else (1 if a < cuts[1] else 2)
        segs.append((a - lo, b - lo, cls))
    return segs


def build_nc(cuts):
    """Build the single-core Bass program (same for all cores; only DRAM
    inputs differ per core).  cuts = [c0, c0+c1] sorted-label boundaries."""
    nc = bacc.Bacc()
    featT = nc.dram_tensor("featT", [C, N], MM_DT, kind="ExternalInput")
    projwT = nc.dram_tensor("projwT", [C, DLOC], MM_DT, kind="ExternalInput")
    cw2t = nc.dram_tensor("cw2t", [PT, NT * K], mybir.dt.float32, kind="ExternalInput")
    p_out = nc.dram_tensor("p_out", [NG, K, N], mybir.dt.float32, kind="ExternalOutput")
    s_out = nc.dram_tensor("s_out", [PT, NT * K], mybir.dt.float32, kind="ExternalOutput")

    # per-chunk segment tables (identical for every t)
    seg_table = [_chunk_segments(j, cuts) for j in range(NJ)]
    ncols = sum(len(s) for s in seg_table)  # accum columns per d-tile

    with tile.TileContext(nc) as tc:
        with (
            tc.tile_pool(name="singles", bufs=1) as singles,
            tc.tile_pool(name="hv", bufs=HV_BUFS) as hvp,
            tc.tile_pool(name="feat", bufs=3) as featp,
            tc.tile_pool(name="spart", bufs=6) as spartp,
            tc.tile_pool(name="pstage", bufs=2) as pstp,
            tc.tile_pool(name="mm1ps", bufs=4, space=MemorySpace.PSUM) as mm1ps,
            tc.tile_pool(name="pps", bufs=2, space=MemorySpace.PSUM) as pps,
        ):
            projw_sb = singles.tile([C, DLOC], MM_DT)
            nc.sync.dma_start(out=projw_sb, in_=projwT[:, :])
            cw2_sb = singles.tile([PT, NT * K], mybir.dt.float32)
            nc.sync.dma_start(out=cw2_sb, in_=cw2t[:, :])
            upd_sb = singles.tile([PT, NT * K], mybir.dt.bfloat16)
            s_sb = singles.tile([PT, NT * K], mybir.dt.float32)
            nc.vector.memset(s_sb, 0.0)

            sidx = 0  # sign-tile counter for ACT/DVE balance
            for g, ts in enumerate(GROUPS):
                hv = {}
                spart = {}
                for t in ts:
                    hv[t] = hvp.tile([PT, N], mybir.dt.bfloat16, tag="hv", name=f"hv{t}")
                    spart[t] = spartp.tile([PT, 40], mybir.dt.float32, tag="sp", name=f"sp{t}")

                # ---- produce: hvT tiles + per-segment sums --------------
                for j in range(NJ):
                    fj = featp.tile([C, NCH], MM_DT, tag="fj")
                    nc.sync.dma_start(out=fj, in_=featT[:, j * NCH:(j + 1) * NCH])
                    for t in ts:
                        ps = mm1ps.tile([PT, NCH], mybir.dt.float32, tag="mm1")
                        nc.tensor.matmul(
                            ps, projw_sb[:, t * PT:(t + 1) * PT], fj,
                            start=True, stop=True,
                        )
                        col0 = sum(len(seg_table[jj]) for jj in range(j))
                        on_act = (sidx % ACT_MOD) != (ACT_MOD - 1)
                        sidx += 1
                        for si, (s0, s1, _cls) in enumerate(seg_table[j]):
                            hv_sl = hv[t][:, j * NCH + s0: j * NCH + s1]
                            acc = spart[t][:, col0 + si: col0 + si + 1]
                            if on_act:
                                nc.scalar.activation(
                                    hv_sl, ps[:, s0:s1],
                                    mybir.ActivationFunctionType.Sign,
                                    accum_out=acc,
                                )
                            else:
                                nc.vector.tensor_scalar(
                                    hv_sl, ps[:, s0:s1], 0.0, 2.0,
                                    mybir.AluOpType.is_ge, mybir.AluOpType.mult,
                                )
                                # op1 is the accum reduction op (add), not elementwise
                                nc.vector.tensor_scalar(
                                    hv_sl, hv_sl, -1.0, None,
                                    mybir.AluOpType.add, mybir.AluOpType.add,
                                    accum_out=acc,
                                )

                # ---- collapse segment partials -> S, u2 -----------------
                # accum columns are in non-decreasing class order
                col_cls = [cls for j in range(NJ) for (_a, _b, cls) in seg_table[j]]
                for t in ts:
                    for k in range(K):
                        idxs = [i for i, cc in enumerate(col_cls) if cc == k]
                        if not idxs:
                            continue
                        a, b = idxs[0], idxs[-1] + 1
                        nc.vector.reduce_sum(
                            s_sb[:, t * K + k: t * K + k + 1],
                            spart[t][:, a:b],
                            axis=mybir.AxisListType.X,
                        )
                    nc.vector.tensor_add(
                        upd_sb[:, t * K:(t + 1) * K],
                        s_sb[:, t * K:(t + 1) * K],
                        cw2_sb[:, t * K:(t + 1) * K],
                    )

                # ---- consume: P2 partial = u2.T @ hvT -------------------
                for jj in range(NJJ):
                    pp = pps.tile([K, PCH], mybir.dt.float32, tag="pp")
                    for i, t in enumerate(ts):
                        for h in range(2):
                            nc.tensor.matmul(
                                pp[:, h * NCH:(h + 1) * NCH],
                                upd_sb[:, t * K:(t + 1) * K],
                                hv[t][:, jj * PCH + h * NCH: jj * PCH + (h + 1) * NCH],
                                start=(i == 0), stop=(i == len(ts) - 1),
                            )
                    pst = pstp.tile([K, PCH], mybir.dt.float32, tag="pst")
                    nc.vector.tensor_copy(pst, pp)
                    nc.sync.dma_start(
                        out=p_out[g, :, jj * PCH:(jj + 1) * PCH], in_=pst
                    )

            nc.sync.dma_start(out=s_out[:, :], in_=s_sb)
    nc.compile()
    return nc


def _prep_inputs(feat_s, proj_w, classify_weights):
    featT = np.ascontiguousarray(feat_s.T).astype(np.float32)  # [128, N]
    in_maps = []
    for core in range(NCORES):
        sl = slice(core * DLOC, (core + 1) * DLOC)
        projwT = np.ascontiguousarray(proj_w[sl].T).astype(np.float32)  # [128, DLOC]
        cw2 = (2.0 * classify_weights[:, sl].astype(np.float32)).T      # [DLOC, 3]
        cw2t = np.ascontiguousarray(
            cw2.reshape(NT, PT, K).transpose(1, 0, 2).reshape(PT, NT * K)
        )
        in_maps.append({"featT": featT, "projwT": projwT, "cw2t": cw2t})
    return in_maps


def kernel(feat, proj_w, classify_weights, labels, _trace=False):
    global LAST_RESULTS
    feat = np.asarray(feat, dtype=np.float32)
    proj_w = np.asarray(proj_w, dtype=np.float32)
    classify_weights = np.asarray(classify_weights, dtype=np.float32)
    labels = np.asarray(labels).astype(np.int64)

    perm = np.argsort(labels, kind="stable")
    feat_s = feat[perm]
    counts = np.bincount(labels, minlength=K)
    cuts = [int(counts[0]), int(counts[0] + counts[1])]

    nc = build_nc(cuts)
    in_maps = _prep_inputs(feat_s, proj_w, classify_weights)
    res = run_bass_kernel_spmd(
        nc, in_maps, list(range(NCORES)), trace=_trace
    )
    LAST_RESULTS = res

    S = np.zeros((K, D), np.float32)
    P2 = np.zeros((K, N), np.float64)
    for core in range(NCORES):
        s_o = np.asarray(res.results[core]["s_out"])          # [PT, NT*K]
        s_full = s_o.reshape(PT, NT, K).transpose(1, 0, 2).reshape(DLOC, K)
        S[:, core * DLOC:(core + 1) * DLOC] = s_full.T
        P2 += np.asarray(res.results[core]["p_out"]).astype(np.float64).sum(axis=0)

    updated = classify_weights + np.float32(LAM) * S          # [K, D] f32
    norms = np.linalg.norm(updated, axis=1)                   # f32-ish norms
    scale = 0.5 / np.maximum(norms, EPS)
    logits_sorted = (P2 * scale[:, None]).T.astype(np.float32)  # [N, K]
    out = np.empty((N, K), np.float32)
    out[perm] = logits_sorted
    return out



# revision 2
# speedup vs baseline: 1.3696x; 1.3696x over previous
"""Trainium2 Bass kernel for the vq_codebook / HDC problem (v2).

Math (reference):
    hv      = sign(feat @ proj_w.T)               [N=16384, D=10000], +-1 (0 -> +1)
    per_cls = segment_sum(hv, labels, K=3)        [3, D]
    updated = classify_weights + 0.5 * per_cls
    protos  = updated / max(||updated||_row, eps)
    logits  = hv @ protos.T                       [N, 3]

Strategy v2 (8 NeuronCores, D-sharded, no collectives):
  * Each core owns DLOC=1250 hyper-dims (10 d-tiles x 125 partitions), all
    N rows; host sorts rows by label so per-class sums become contiguous
    range sums along the free axis.
  * hv is stored as step(z) in {0,1} fp8e4 (16KB/partition per d-tile), so
    ALL 10 d-tiles stay SBUF-resident: feat is read exactly once per core
    and hv never touches HBM.  sign = 2*step - 1 is fixed up algebraically
    on the host (counts and row-sums are known there).
  * Phase A (encode): z = projw_loc.T-tiles @ featT in [d, n] layout via
    fp32r matmuls into 4-bank PSUM super-tiles [125, 2048]; ONE drain op
    per (d-tile, n-group) converts z -> step fp8 with a fused per-segment
    accumulation (accum_out).  Drains are split between DVE (is_ge) and
    ACT (Sigmoid(2^20 z)) with a greedy balance on the documented cost
    models; the engines run concurrently on different PSUM banks.
  * Phase B: u = (cw - 0.5*count) + step-sums == reference `updated`
    exactly; phase-C stationary = fp8(u/2) + fp8 residual per d-tile.
  * Phase C (consume): P = (u_q/2) @ step partials via fp8 DoubleRow
    matmuls -- pairs of d-tiles form a 250-deep contraction at 2 moving
    elements/cycle; 5 pairs accumulate in PSUM; [6, 2048] drains alternate
    DVE/ACT; DMA out.
  * Host: logits = (4*P - rowsum(u_q)) / max(||u||, eps), un-permuted.
"""

import os
import sys

sys.path.insert(0, "/opt/trn_rl_repo")
os.environ.setdefault("MYCRO_LOCAL_CACHE", "1")

import numpy as np

import concourse.bass as bass
import concourse.tile as tile
from concourse import bacc
from concourse import mybir
from concourse.bass import MemorySpace
from concourse.bass_utils import run_bass_kernel_spmd

# ---------------------------------------------------------------- constants
N, C, D, K = 16384, 128, 10000, 3
NCORES = 8
DLOC = D // NCORES          # 1250 hyper-dims per core
PT = 125                    # partitions per d-tile
NT = DLOC // PT             # 10 d-tiles per core
NPAIR = NT // 2             # 5 DoubleRow pairs
G4 = 2048                   # phase-A drain super-chunk (4 PSUM banks)
NG = N // G4                # 8 groups
MMC = 512                   # encode matmul moving chunk (fp32 max)
PC = 512                    # phase-C matmul out chunk (1 PSUM bank)
PCG = 2048                  # phase-C drain super-chunk
SIG_SCALE = 1048576.0       # 2^20: step(z) ~= Sigmoid(SIG_SCALE * z)
ACOLS = 12                  # accum columns reserved per d-tile

MM_DT = mybir.dt.float32r
FP8 = mybir.dt.float8e4
F32 = mybir.dt.float32
EPS = 1e-12

LAST_RESULTS = None         # BassKernelResults of the most recent run


def _subranges(cuts):
    """Ordered (g, s0, s1, cls) sub-ranges: each phase-A group [g*G4,(g+1)*G4)
    split at the sorted-label cut points so every range is single-class."""
    subs = []
    for g in range(NG):
        lo, hi = g * G4, (g + 1) * G4
        pts = [lo] + [c for c in cuts if lo < c < hi] + [hi]
        for a, b in zip(pts[:-1], pts[1:]):
            cls = 0 if a < cuts[0] else (1 if a < cuts[1] else 2)
            subs.append((g, a - lo, b - lo, cls))
    return subs


def _engine_plan(subs):
    """Greedy DVE/ACT split of the NT x len(subs) drain ops, in issue order,
    balancing the documented cost models (DVE (120+FD)/0.96, ACT (172+FD)/1.2)."""
    eng = {}
    tv = ta = 0.0
    for g in range(NG):
        gsubs = [s for s in subs if s[0] == g]
        for t in range(NT):
            for (_g, s0, s1, _cls) in gsubs:
                fd = float(s1 - s0)
                cv = (120.0 + fd) / 0.96
                ca = (172.0 + fd) / 1.2
                if tv + cv <= ta + ca:
                    tv += cv
                    eng[(g, t, s0)] = "V"
                else:
                    ta += ca
                    eng[(g, t, s0)] = "A"
    return eng


def build_nc(cuts):
    """Single-core Bass program (identical on all cores; only DRAM inputs
    differ).  cuts = [c0, c0+c1] sorted-label class boundaries."""
    subs = _subranges(cuts)
    ncols = len(subs)
    assert ncols <= ACOLS, ncols
    eng = _engine_plan(subs)

    # class -> accum-column range (same structure for every d-tile)
    col_cls = [cls for (_g, _s0, _s1, cls) in subs]
    crange = []
    for k in range(K):
        idx = [i for i, cc in enumerate(col_cls) if cc == k]
        assert idx, f"class {k} empty"
        assert idx == list(range(idx[0], idx[-1] + 1))
        crange.append((idx[0], idx[-1] + 1))

    nc = bacc.Bacc()
    featT = nc.dram_tensor("featT", [C, N], MM_DT, kind="ExternalInput")
    projwT = nc.dram_tensor("projwT", [C, DLOC], MM_DT, kind="ExternalInput")
    cwadj = nc.dram_tensor("cwadj", [PT, NT * K], F32, kind="ExternalInput")
    p_out = nc.dram_tensor("p_out", [2 * K, N], F32, kind="ExternalOutput")
    u_out = nc.dram_tensor("u_out", [PT, NT * K], F32, kind="ExternalOutput")

    with tile.TileContext(nc) as tc:
        with tc.tile_pool(name="singles", bufs=1) as singles:
            projw_sb = singles.tile([C, DLOC], MM_DT)
            nc.sync.dma_start(out=projw_sb, in_=projwT[:, :])
            cwadj_sb = singles.tile([PT, NT * K], F32)
            nc.sync.dma_start(out=cwadj_sb, in_=cwadj[:, :])
            hv = [
                singles.tile([PT, 2, N], FP8, name=f"hv{pr}")
                for pr in range(NPAIR)
            ]
            sacc = singles.tile([PT, NT * ACOLS], F32)
            ssum = singles.tile([PT, NT * K], F32)
            u_sb = singles.tile([PT, NT * K], F32)
            a32 = singles.tile([PT, NT * K], F32)
            stat = [
                singles.tile([PT, 2, 16], FP8, name=f"stat{pr}")
                for pr in range(NPAIR)
            ]

            # ---- phase A: encode + step + fused segment sums ------------
            with (
                tc.tile_pool(name="featp", bufs=2) as featp,
                tc.tile_pool(name="zp", bufs=2, space="PSUM") as zp,
            ):
                for g in range(NG):
                    fj = featp.tile([C, G4], MM_DT, tag="fj")
                    nc.sync.dma_start(
                        out=fj, in_=featT[:, g * G4:(g + 1) * G4]
                    )
                    gsubs = [
                        (s0, s1, ci)
                        for ci, (gg, s0, s1, _c) in enumerate(subs)
                        if gg == g
                    ]
                    for t in range(NT):
                        z = zp.tile([PT, G4], F32, tag="z")
                        for c4 in range(G4 // MMC):
                            nc.tensor.matmul(
                                z[:, c4 * MMC:(c4 + 1) * MMC],
                                projw_sb[:, t * PT:(t + 1) * PT],
                                fj[:, c4 * MMC:(c4 + 1) * MMC],
                                start=True, stop=True,
                            )
                        pr, ko = divmod(t, 2)
                        for (s0, s1, ci) in gsubs:
                            dst = hv[pr][:, ko, g * G4 + s0:g * G4 + s1]
                            acc = sacc[:, t * ACOLS + ci:t * ACOLS + ci + 1]
                            if eng[(g, t, s0)] == "A":
                                nc.scalar.activation(
                                    dst, z[:, s0:s1],
                                    mybir.ActivationFunctionType.Sigmoid,
                                    scale=SIG_SCALE,
                                    accum_out=acc,
                                )
                            else:
                                nc.vector.tensor_scalar(
                                    dst, z[:, s0:s1], 0.0, None,
                                    mybir.AluOpType.is_ge,
                                    mybir.AluOpType.add,
                                    accum_out=acc,
                                )

            # ---- phase B: u = cwadj + step-sums; fp8 hi+res stationary --
            sacc3 = sacc.rearrange("p (t c) -> p t c", c=ACOLS)
            ssum3 = ssum.rearrange("p (t k) -> p t k", k=K)
            for k in range(K):
                a, b = crange[k]
                nc.vector.reduce_sum(
                    ssum3[:, :, k:k + 1], sacc3[:, :, a:b],
                    axis=mybir.AxisListType.X,
                )
            nc.vector.tensor_add(u_sb, ssum, cwadj_sb)
            nc.sync.dma_start(out=u_out[:, :], in_=u_sb)
            for pr in range(NPAIR):
                u_v = u_sb[:, pr * 6:(pr + 1) * 6].rearrange(
                    "p (b c) -> p b c", b=2
                )
                a32_v = a32[:, pr * 6:(pr + 1) * 6].rearrange(
                    "p (b c) -> p b c", b=2
                )
                nc.vector.tensor_scalar(
                    stat[pr][:, :, 0:K], u_v, 0.5, None,
                    mybir.AluOpType.mult,
                )
                nc.vector.tensor_copy(a32_v, stat[pr][:, :, 0:K])
                nc.vector.scalar_tensor_tensor(
                    stat[pr][:, :, K:2 * K], u_v, 0.5, a32_v,
                    mybir.AluOpType.mult, mybir.AluOpType.subtract,
                )

            # ---- phase C: P partials via fp8 DoubleRow matmuls ----------
            with (
                tc.tile_pool(name="pp", bufs=2, space="PSUM") as ppp,
                tc.tile_pool(name="pstage", bufs=2) as pstp,
            ):
                for gc in range(N // PCG):
                    pq = ppp.tile([2 * K, PCG], F32, tag="pq")
                    for c4 in range(PCG // PC):
                        base = gc * PCG + c4 * PC
                        for pr in range(NPAIR):
                            nc.tensor.matmul(
                                pq[:, c4 * PC:(c4 + 1) * PC],
                                stat[pr][:, :, 0:2 * K],
                                hv[pr][:, :, base:base + PC],
                                start=(pr == 0), stop=(pr == NPAIR - 1),
                                perf_mode=mybir.MatmulPerfMode.DoubleRow,
                            )
                    pst = pstp.tile([2 * K, PCG], F32, tag="pst")
                    if gc % 2 == 0:
                        nc.vector.tensor_copy(pst, pq)
                    else:
                        nc.scalar.copy(pst, pq)
                    nc.sync.dma_start(
                        out=p_out[:, gc * PCG:(gc + 1) * PCG], in_=pst
                    )
    nc.compile()
    return nc


def _prep_inputs(feat_s, proj_w, classify_weights, counts):
    featT = np.ascontiguousarray(feat_s.T).astype(np.float32)  # [128, N]
    cadj = classify_weights.astype(np.float32) \
        - 0.5 * counts[:, None].astype(np.float32)             # [K, D]
    in_maps = []
    for core in range(NCORES):
        sl = slice(core * DLOC, (core + 1) * DLOC)
        projwT = np.ascontiguousarray(proj_w[sl].T).astype(np.float32)
        ca = cadj[:, sl].T                                     # [DLOC, K]
        ca_t = np.ascontiguousarray(
            ca.reshape(NT, PT, K).transpose(1, 0, 2).reshape(PT, NT * K)
        ).astype(np.float32)
        in_maps.append({"featT": featT, "projwT": projwT, "cwadj": ca_t})
    return in_maps


def _assemble(results, perm):
    """Host: gather per-core u/P, undo the step->sign affine, normalize."""
    fp8np = mybir.dt.np(FP8)
    P = np.zeros((K, N), np.float64)
    rowsum_uq = np.zeros(K, np.float64)
    U = np.zeros((K, D), np.float32)
    for core in range(NCORES):
        r = results[core]
        u = np.asarray(r["u_out"])                             # [PT, NT*K]
        u_full = u.reshape(PT, NT, K).transpose(1, 0, 2).reshape(DLOC, K)
        U[:, core * DLOC:(core + 1) * DLOC] = u_full.T
        # replicate the device fp8 hi+res quantization exactly
        a32f = (0.5 * u_full).astype(fp8np).astype(np.float32)
        b32f = (0.5 * u_full - a32f).astype(fp8np).astype(np.float32)
        rowsum_uq += 2.0 * (a32f + b32f).astype(np.float64).sum(axis=0)
        p6 = np.asarray(r["p_out"]).astype(np.float64)         # [6, N]
        P += p6[0:K] + p6[K:2 * K]
    # logits2[k,n] = sum_d u_q[d,k] * (2*step - 1) = 4*P - rowsum(u_q)
    L2 = 4.0 * P - rowsum_uq[:, None]
    norms = np.linalg.norm(U.astype(np.float64), axis=1)
    logits_sorted = (L2 / np.maximum(norms, EPS)[:, None]).T.astype(np.float32)
    out = np.empty((N, K), np.float32)
    out[perm] = logits_sorted
    return out


def kernel(feat, proj_w, classify_weights, labels, _trace=False):
    global LAST_RESULTS
    feat = np.asarray(feat, dtype=np.float32)
    proj_w = np.asarray(proj_w, dtype=np.float32)
    classify_weights = np.asarray(classify_weights, dtype=np.float32)
    labels = np.asarray(labels).astype(np.int64)

    perm = np.argsort(labels, kind="stable")
    feat_s = feat[perm]
    counts = np.bincount(labels, minlength=K)
    cuts = [int(counts[0]), int(counts[0] + counts[1])]

    nc = build_nc(cuts)
    in_maps = _prep_inputs(feat_s, proj_w, classify_weights, counts)
    res = run_bass_kernel_spmd(nc, in_maps, list(range(NCORES)), trace=_trace)
    LAST_RESULTS = res
    return _assemble(res.results, perm)


# revision 6
# speedup vs baseline: 1.4181x; 1.0354x over previous
"""Trainium2 Bass kernel for the vq_codebook / HDC problem (v3).

Math (reference):
    hv      = sign(feat @ proj_w.T)               [N=16384, D=10000], +-1 (0 -> +1)
    per_cls = segment_sum(hv, labels, K=3)        [3, D]
    updated = classify_weights + 0.5 * per_cls
    protos  = updated / max(||updated||_row, eps)
    logits  = hv @ protos.T                       [N, 3]

Strategy v3 (8 NeuronCores, D-sharded, no collectives):
  * Each core owns DLOC=1250 hyper-dims (10 d-tiles x 125 partitions), all
    N rows; host sorts rows by label so per-class sums become contiguous
    range sums along the free axis.
  * hv is stored as step(z) in {0,1} fp8e4 (16KB/partition per d-tile), so
    ALL 10 d-tiles stay SBUF-resident: feat is read exactly once per core
    and hv never touches HBM.  sign = 2*step - 1 is fixed up algebraically
    on the host.
  * Phase A (encode): z = projw_loc.T-tiles @ featT in [d, n] layout via
    fp16 matmuls (16-bit moving operand streams at 1 col/cycle vs 2 for
    fp32r; end-to-end rel err ~5e-3, measured) into 4-bank PSUM
    super-tiles [125, 2048]; ONE drain op per (d-tile, n-group) converts
    z -> step fp8 with fused per-segment accumulation (accum_out), split
    between DVE (is_ge) and ACT (Sigmoid(2^20 z)) via a greedy balance on
    trace-fitted cost models.  The drain pool is the phase-A bottleneck
    (~95us); the encode PE stream (~75us) hides under it.
  * Phase B: u = (cw - 0.5*count) + step-sums == reference `updated`
    exactly; phase-C stationary = fp8(u/2) + fp8 residual per d-tile.
  * Phase C (consume): P = (u_q/2) @ step via plain-fp8 matmuls packed 4x
    into the PE array with column tiling (out partitions 6 of 128, so 4
    concurrent col-groups each own 2-3 d-tiles and stream their own
    moving operand; accumulation groups are per col-group).
  * Host: logits = (4*P - rowsum(u_q)) / max(||u||, eps), un-permuted.
"""

import os
import sys

sys.path.insert(0, "/opt/trn_rl_repo")
os.environ.setdefault("MYCRO_LOCAL_CACHE", "1")

import numpy as np

import concourse.bass as bass
import concourse.tile as tile
from concourse import bacc
from concourse import mybir
from concourse.bass import MemorySpace
from concourse.bass_utils import run_bass_kernel_spmd

# ---------------------------------------------------------------- constants
N, C, D, K = 16384, 128, 10000, 3
NCORES = 8
DLOC = D // NCORES          # 1250 hyper-dims per core
PT = 125                    # partitions per d-tile
NT = DLOC // PT             # 10 d-tiles per core
G4 = 2048                   # phase-A drain super-chunk (4 PSUM banks)
NG = N // G4                # 8 groups
MMC = 512                   # encode matmul chunk (PSUM bank = 512 fp32)
PC = 512                    # phase-C matmul out chunk (1 PSUM bank)
PCG = 2048                  # phase-C drain super-chunk
SIG_SCALE = 1048576.0       # 2^20: step(z) ~= Sigmoid(SIG_SCALE * z)
ACOLS = 12                  # accum columns reserved per d-tile

# phase-C column tiling: d-tile -> col group (3/3/2/2), round-robin issue
GID = [0, 0, 0, 1, 1, 1, 2, 2, 3, 3]
RR = [0, 3, 6, 8, 1, 4, 7, 9, 2, 5]
FIRST = {0: 0, 1: 3, 2: 6, 3: 8}
LAST = {0: 2, 1: 5, 2: 7, 3: 9}
SCW = 32                    # stationary cols (zero-padded past 2K so every
                            # PSUM partition in a col group is written)

MM_DT = mybir.dt.float16
FP8 = mybir.dt.float8e4
F32 = mybir.dt.float32
EPS = 1e-12

LAST_RESULTS = None         # BassKernelResults of the most recent run


def _subranges(cuts):
    """Ordered (g, s0, s1, cls) sub-ranges: each phase-A group [g*G4,(g+1)*G4)
    split at the sorted-label cut points so every range is single-class."""
    subs = []
    for g in range(NG):
        lo, hi = g * G4, (g + 1) * G4
        pts = [lo] + [c for c in cuts if lo < c < hi] + [hi]
        for a, b in zip(pts[:-1], pts[1:]):
            cls = 0 if a < cuts[0] else (1 if a < cuts[1] else 2)
            subs.append((g, a - lo, b - lo, cls))
    return subs


def _engine_plan(subs):
    """Greedy DVE/ACT split of the NT x len(subs) drain ops in issue order,
    using trace-fitted per-op costs (ns): DVE ~ 200+1.10*FD (incl. queue
    extras), ACT ~ 550+0.88*FD (ACTIVATE + accum-read + sem)."""
    eng = {}
    tv = ta = 0.0
    for g in range(NG):
        gsubs = [s for s in subs if s[0] == g]
        for t in range(NT):
            for (_g, s0, s1, _cls) in gsubs:
                fd = float(s1 - s0)
                cv = 200.0 + 1.10 * fd
                ca = 550.0 + 0.88 * fd
                if tv + cv <= ta + ca:
                    tv += cv
                    eng[(g, t, s0)] = "V"
                else:
                    ta += ca
                    eng[(g, t, s0)] = "A"
    return eng


def build_nc(cuts):
    """Single-core Bass program (identical on all cores; only DRAM inputs
    differ).  cuts = [c0, c0+c1] sorted-label class boundaries."""
    subs = _subranges(cuts)
    ncols = len(subs)
    assert ncols <= ACOLS, ncols
    eng = _engine_plan(subs)

    # class -> accum-column range (same structure for every d-tile)
    col_cls = [cls for (_g, _s0, _s1, cls) in subs]
    crange = []
    for k in range(K):
        idx = [i for i, cc in enumerate(col_cls) if cc == k]
        assert idx, f"class {k} empty"
        assert idx == list(range(idx[0], idx[-1] + 1))
        crange.append((idx[0], idx[-1] + 1))

    nc = bacc.Bacc()
    featT = nc.dram_tensor("featT", [C, N], MM_DT, kind="ExternalInput")
    projwT = nc.dram_tensor("projwT", [C, DLOC], MM_DT, kind="ExternalInput")
    cwadj = nc.dram_tensor("cwadj", [PT, NT * K], F32, kind="ExternalInput")
    p_out = nc.dram_tensor("p_out", [4, 2 * K, N], F32, kind="ExternalOutput")
    u_out = nc.dram_tensor("u_out", [PT, NT * K], F32, kind="ExternalOutput")

    with tile.TileContext(nc) as tc:
        with tc.tile_pool(name="singles", bufs=1) as singles:
            projw_sb = singles.tile([C, DLOC], MM_DT)
            nc.sync.dma_start(out=projw_sb, in_=projwT[:, :])
            cwadj_sb = singles.tile([PT, NT * K], F32)
            nc.sync.dma_start(out=cwadj_sb, in_=cwadj[:, :])
            hv = [
                singles.tile([PT, N], FP8, name=f"hv{t}")
                for t in range(NT)
            ]
            sacc = singles.tile([PT, NT * ACOLS], F32)
            ssum = singles.tile([PT, NT * K], F32)
            u_sb = singles.tile([PT, NT * K], F32)
            a32 = singles.tile([PT, NT * K], F32)
            stat = singles.tile([PT, NT * SCW], FP8)
            nc.vector.memset(stat, 0.0)

            # ---- phase A: encode + step + fused segment sums ------------
            with (
                tc.tile_pool(name="featp", bufs=2) as featp,
                tc.tile_pool(name="zp", bufs=2, space="PSUM") as zp,
            ):
                for g in range(NG):
                    fj = featp.tile([C, G4], MM_DT, tag="fj")
                    nc.sync.dma_start(
                        out=fj, in_=featT[:, g * G4:(g + 1) * G4]
                    )
                    gsubs = [
                        (s0, s1, ci)
                        for ci, (gg, s0, s1, _c) in enumerate(subs)
                        if gg == g
                    ]
                    for t in range(NT):
                        z = zp.tile([PT, G4], F32, tag="z")
                        for c4 in range(G4 // MMC):
                            nc.tensor.matmul(
                                z[:, c4 * MMC:(c4 + 1) * MMC],
                                projw_sb[:, t * PT:(t + 1) * PT],
                                fj[:, c4 * MMC:(c4 + 1) * MMC],
                                start=True, stop=True,
                            )
                        for (s0, s1, ci) in gsubs:
                            dst = hv[t][:, g * G4 + s0:g * G4 + s1]
                            acc = sacc[:, t * ACOLS + ci:t * ACOLS + ci + 1]
                            if eng[(g, t, s0)] == "A":
                                nc.scalar.activation(
                                    dst, z[:, s0:s1],
                                    mybir.ActivationFunctionType.Sigmoid,
                                    scale=SIG_SCALE,
                                    accum_out=acc,
                                )
                            else:
                                nc.vector.tensor_scalar(
                                    dst, z[:, s0:s1], 0.0, None,
                                    mybir.AluOpType.is_ge,
                                    mybir.AluOpType.add,
                                    accum_out=acc,
                                )

            # ---- phase B: u = cwadj + step-sums; fp8 hi+res stationary --
            sacc3 = sacc.rearrange("p (t c) -> p t c", c=ACOLS)
            ssum3 = ssum.rearrange("p (t k) -> p t k", k=K)
            for k in range(K):
                a, b = crange[k]
                nc.vector.reduce_sum(
                    ssum3[:, :, k:k + 1], sacc3[:, :, a:b],
                    axis=mybir.AxisListType.X,
                )
            nc.vector.tensor_add(u_sb, ssum, cwadj_sb)
            nc.sync.dma_start(out=u_out[:, :], in_=u_sb)
            u3 = u_sb.rearrange("p (t k) -> p t k", k=K)
            a32_3 = a32.rearrange("p (t k) -> p t k", k=K)
            stat3 = stat.rearrange("p (t c) -> p t c", c=SCW)
            nc.vector.tensor_scalar(
                stat3[:, :, 0:K], u3, 0.5, None, mybir.AluOpType.mult,
            )
            nc.vector.tensor_copy(a32_3, stat3[:, :, 0:K])
            nc.vector.scalar_tensor_tensor(
                stat3[:, :, K:2 * K], u3, 0.5, a32_3,
                mybir.AluOpType.mult, mybir.AluOpType.subtract,
            )

            # ---- phase C: P partials, 4x column-tiled fp8 matmuls -------
            with (
                tc.tile_pool(name="pp", bufs=2, space="PSUM") as ppp,
                tc.tile_pool(name="pstage", bufs=2) as pstp,
            ):
                for gc in range(N // PCG):
                    pq = ppp.tile([128, PCG], F32, tag="pq")
                    for c4 in range(PCG // PC):
                        cs = slice(c4 * PC, (c4 + 1) * PC)
                        base = gc * PCG + c4 * PC
                        for t in RR:
                            gi = GID[t]
                            nc.tensor.matmul(
                                pq[32 * gi:32 * gi + 32, cs],
                                stat[:, t * SCW:(t + 1) * SCW],
                                hv[t][:, base:base + PC],
                                start=(t == FIRST[gi]),
                                stop=(t == LAST[gi]),
                                tile_position=(0, 32 * gi),
                                # the sim's group checker is zero-region
                                # (partition-blind); has_written is
                                # per-element so col groups are independent
                                skip_group_check=True,
                            )
                    pst = pstp.tile([128, PCG], F32, tag="pst")
                    if gc % 2 == 0:
                        nc.vector.tensor_copy(pst, pq)
                    else:
                        nc.scalar.copy(pst, pq)
                    for gi in range(4):
                        nc.sync.dma_start(
                            out=p_out[gi, :, gc * PCG:(gc + 1) * PCG],
                            in_=pst[32 * gi:32 * gi + 2 * K, :],
                        )
    nc.compile()
    return nc


def _prep_inputs(feat_s, proj_w, classify_weights, counts):
    featT = np.ascontiguousarray(feat_s.T).astype(np.float16)  # [128, N]
    cadj = classify_weights.astype(np.float32) \
        - 0.5 * counts[:, None].astype(np.float32)             # [K, D]
    in_maps = []
    for core in range(NCORES):
        sl = slice(core * DLOC, (core + 1) * DLOC)
        projwT = np.ascontiguousarray(proj_w[sl].T).astype(np.float16)
        ca = cadj[:, sl].T                                     # [DLOC, K]
        ca_t = np.ascontiguousarray(
            ca.reshape(NT, PT, K).transpose(1, 0, 2).reshape(PT, NT * K)
        ).astype(np.float32)
        in_maps.append({"featT": featT, "projwT": projwT, "cwadj": ca_t})
    return in_maps


def _assemble(results, perm):
    """Host: gather per-core u/P, undo the step->sign affine, normalize."""
    fp8np = mybir.dt.np(FP8)
    P = np.zeros((K, N), np.float64)
    rowsum_uq = np.zeros(K, np.float64)
    U = np.zeros((K, D), np.float32)
    for core in range(NCORES):
        r = results[core]
        u = np.asarray(r["u_out"])                             # [PT, NT*K]
        u_full = u.reshape(PT, NT, K).transpose(1, 0, 2).reshape(DLOC, K)
        U[:, core * DLOC:(core + 1) * DLOC] = u_full.T
        # replicate the device fp8 hi+res quantization exactly
        a32f = (0.5 * u_full).astype(fp8np).astype(np.float32)
        b32f = (0.5 * u_full - a32f).astype(fp8np).astype(np.float32)
        rowsum_uq += 2.0 * (a32f + b32f).astype(np.float64).sum(axis=0)
        p6 = np.asarray(r["p_out"]).astype(np.float64)         # [4, 6, N]
        P += (p6[:, 0:K] + p6[:, K:2 * K]).sum(axis=0)
    # logits2[k,n] = sum_d u_q[d,k] * (2*step - 1) = 4*P - rowsum(u_q)
    L2 = 4.0 * P - rowsum_uq[:, None]
    norms = np.linalg.norm(U.astype(np.float64), axis=1)
    logits_sorted = (L2 / np.maximum(norms, EPS)[:, None]).T.astype(np.float32)
    out = np.empty((N, K), np.float32)
    out[perm] = logits_sorted
    return out


def kernel(feat, proj_w, classify_weights, labels, _trace=False):
    global LAST_RESULTS
    feat = np.asarray(feat, dtype=np.float32)
    proj_w = np.asarray(proj_w, dtype=np.float32)
    classify_weights = np.asarray(classify_weights, dtype=np.float32)
    labels = np.asarray(labels).astype(np.int64)

    perm = np.argsort(labels, kind="stable")
    feat_s = feat[perm]
    counts = np.bincount(labels, minlength=K)
    cuts = [int(counts[0]), int(counts[0] + counts[1])]

    nc = build_nc(cuts)
    in_maps = _prep_inputs(feat_s, proj_w, classify_weights, counts)
    res = run_bass_kernel_spmd(nc, in_maps, list(range(NCORES)), trace=_trace)
    LAST_RESULTS = res
    return _assemble(res.results, perm)
